# revision 11
# baseline (speedup 1.0000x reference)
"""Trainium2 Bass kernel: batched pairwise Hessian blocks (Coords2Stress).

For each example b:  out[b, 3i+a, 3j+c] = -sep_a*sep_c/(|sep|^2+eps) for the
off-diagonal atom blocks (masked to the valid atom count), with the 3x3
diagonal blocks overwritten by the negative row-sums.

Strategy: the output is the dominant cost (B * (3N)^2 fp32 = 302MB), so the
kernel is write-bandwidth bound.  Work is split into (example, 128-atom
row-tile) items; only items with any valid rows exist, and each item only
computes/writes columns up to a per-slot static width (>= 3*num_atoms of its
example).  Items are load-balanced across the 8 cores into "slots" so every
core executes the identical (SPMD) program.  Unwritten output stays zero
(run_bass_kernel_spmd pre-zeroes ExternalOutput buffers) and the host
scatters the per-item blocks back into the full [B, 3N, 3N] array.
"""

import os
import sys

import numpy as np

for _p in ("/opt/trn_rl_repo", "/root/.axon_site/_ro/trn_rl_repo"):
    if os.path.isdir(_p) and _p not in sys.path:
        sys.path.insert(0, _p)

import concourse.bass as bass
import concourse.bacc as bacc
import concourse.tile as tile
from concourse import mybir
from concourse.bass_utils import run_bass_kernel_spmd

N_CORES = 8
P = 128  # atoms per work item == SBUF partitions
EPS = 1e-5
F32 = mybir.dt.float32
I32 = mybir.dt.int32
OP = mybir.AluOpType


def _plan(num_atoms):
    """Work items -> slots.  Item (b, t) covers atoms [128t, 128t+128) of
    example b; its weight is the j-extent (in atoms) the item must compute:
    max(num_atoms[b], 128*(t+1)) -- the latter keeps the diagonal window in
    range for partial tiles.  Returns [(slot_width, [(weight, b, t), ...])]."""
    items = []
    for b, na in enumerate(num_atoms):
        na = int(na)
        if na <= 0:
            continue
        nt = -(-na // P)
        for t in range(nt):
            items.append((max(na, P * (t + 1)), b, t))
    items.sort(key=lambda x: (-x[0], x[1], x[2]))
    nslot = max(1, -(-len(items) // N_CORES))
    slots = []
    for k in range(nslot):
        chunk = items[k * N_CORES:(k + 1) * N_CORES]
        slots.append((chunk[0][0], chunk))
    return slots


def _offsets(widths):
    cf, cm, oo = [], [], []
    a = b = c = 0
    for w in widths:
        cf.append(a)
        cm.append(b)
        oo.append(c)
        a += 3 * w
        b += w
        c += 384 * 3 * w
    return cf, cm, oo, a, b, c


def _build(widths):
    """Emit + compile the SPMD program for the given per-slot widths."""
    K = len(widths)
    Wmax = max(widths)
    cf_off, cm_off, out_off, cf_len, cm_len, out_len = _offsets(widths)
    AUXW = 3 * K + K

    nc = bacc.Bacc("TRN2", target_bir_lowering=False, debug=False)
    d_cf = nc.dram_tensor("cf", [cf_len], F32, kind="ExternalInput").ap()
    d_cm = nc.dram_tensor("cm", [cm_len], F32, kind="ExternalInput").ap()
    d_aux = nc.dram_tensor("aux", [P, AUXW], F32, kind="ExternalInput").ap()
    d_out = nc.dram_tensor("out", [out_len], F32, kind="ExternalOutput").ap()
    d_dg = nc.dram_tensor("dg", [K, P, 9], F32, kind="ExternalOutput").ap()

    with tile.TileContext(nc) as tc:
        with (
            tc.tile_pool(name="const", bufs=1) as constp,
            tc.tile_pool(name="p0", bufs=1) as p0p,
            tc.tile_pool(name="bc", bufs=1) as bcp,
            tc.tile_pool(name="cmb", bufs=2) as cmbp,
            tc.tile_pool(name="s", bufs=2) as sp,
            tc.tile_pool(name="s2", bufs=1) as s2p,
            tc.tile_pool(name="mid", bufs=1) as midp,
            tc.tile_pool(name="rm", bufs=2) as rmp,
            tc.tile_pool(name="g", bufs=2) as gp,
            tc.tile_pool(name="dac", bufs=2) as dacp,
            tc.tile_pool(name="row", bufs=2) as rowp,
        ):
            aux = constp.tile([P, AUXW], F32)
            nc.scalar.dma_start(out=aux[:], in_=d_aux)

            for k, w in enumerate(widths):
                ct = aux[:, 3 * k: 3 * k + 3]                        # [P,3]
                rv = aux[:, 3 * K + k: 3 * K + k + 1]                # [P,1]

                cf0 = p0p.tile([1, 3 * Wmax], F32, tag="cf0")
                cm0 = p0p.tile([1, Wmax], F32, tag="cm0")
                nc.scalar.dma_start(
                    out=cf0[:1, :3 * w],
                    in_=d_cf[cf_off[k]: cf_off[k] + 3 * w].unsqueeze(0))
                nc.scalar.dma_start(
                    out=cm0[:1, :w],
                    in_=d_cm[cm_off[k]: cm_off[k] + w].unsqueeze(0))

                cb = bcp.tile([P, 3 * Wmax], F32, tag="cb")
                cmb = cmbp.tile([P, Wmax], F32, tag="cmb")
                nc.gpsimd.partition_broadcast(cb[:, :3 * w], cf0[:1, :3 * w])
                nc.gpsimd.partition_broadcast(cmb[:, :w], cm0[:1, :w])

                # s[p, 3j+c] = ct[p,c] - cb[3j+c]   (= c_i - c_j)
                s = sp.tile([P, 3 * Wmax], F32, tag="s")
                s3 = s[:, :3 * w].rearrange("p (j c) -> p j c", c=3)
                cb3 = cb[:, :3 * w].rearrange("p (j c) -> p j c", c=3)
                ct_b = ct.unsqueeze(1).broadcast_to([P, w, 3])
                nc.vector.scalar_tensor_tensor(
                    s3, cb3, -1.0, ct_b, OP.mult, OP.add)

                # d2e = sum_c s^2 + eps ; r0 = 1/d2e (unmasked distances)
                s2 = s2p.tile([P, 3 * Wmax], F32, tag="s2")
                nc.scalar.square(s2[:, :3 * w], s[:, :3 * w])
                s23 = s2[:, :3 * w].rearrange("p (j c) -> p j c", c=3)
                a1 = midp.tile([P, Wmax], F32, tag="a1")
                nc.vector.scalar_tensor_tensor(
                    a1[:, :w], s23[:, :, 0], 0.0, s23[:, :, 1], OP.add, OP.add)
                d2e = midp.tile([P, Wmax], F32, tag="d2e")
                nc.vector.scalar_tensor_tensor(
                    d2e[:, :w], s23[:, :, 2], float(EPS), a1[:, :w],
                    OP.add, OP.add)
                r0 = midp.tile([P, Wmax], F32, tag="r0")
                nc.vector.reciprocal(r0[:, :w], d2e[:, :w])

                # rm = (-colmask * rowvalid) / d2e   (cm input is negated)
                rm = rmp.tile([P, Wmax], F32, tag="rm")
                nc.vector.scalar_tensor_tensor(
                    rm[:, :w], cmb[:, :w], rv, r0[:, :w], OP.mult, OP.mult)

                # row[p, a, j, c] = s_c * (s_a * rm) = -sep_a*sep_c*m/d2
                # accumulate per-(a,c) row sums for the diagonal blocks
                row = rowp.tile([P, 9 * Wmax], F32, tag="row")
                row4 = row[:, :9 * w].rearrange("p (a j c) -> p a j c",
                                                a=3, c=3)
                dac = dacp.tile([P, 16], F32, tag="dac")
                for a in range(3):
                    g = gp.tile([P, Wmax], F32, tag="g")
                    nc.vector.scalar_tensor_tensor(
                        g[:, :w], s3[:, :, a], 0.0, rm[:, :w],
                        OP.bypass, OP.mult)
                    for c in range(3):
                        nc.vector.scalar_tensor_tensor(
                            row4[:, a, :, c], s3[:, :, c], 0.0, g[:, :w],
                            OP.bypass, OP.mult,
                            accum_out=dac[:, 3 * a + c: 3 * a + c + 1])

                # row sums out to the host, which writes the diagonal blocks
                nc.scalar.dma_start(out=d_dg[k], in_=dac[:, 0:9])
                dro = (d_out[out_off[k]: out_off[k] + 384 * 3 * w]
                       .rearrange("(p a n) -> p a n", p=P, a=3))
                nc.sync.dma_start(
                    out=dro,
                    in_=row[:, :9 * w].rearrange("p (a n) -> p a n", a=3))
    nc.compile()
    return nc


def _pack(coords, num_atoms, slots):
    """Per-core input arrays for the SPMD program."""
    B = coords.shape[0]
    N = coords.shape[1] // 3
    widths = [s[0] for s in slots]
    K = len(slots)
    AUXW = 3 * K + K
    cf_off, cm_off, out_off, cf_len, cm_len, out_len = _offsets(widths)
    c3 = coords.reshape(B, N, 3)
    pidx = np.arange(P)

    in_maps = []
    for _ in range(N_CORES):
        in_maps.append({
            "cf": np.zeros(cf_len, np.float32),
            "cm": np.zeros(cm_len, np.float32),
            "aux": np.zeros((P, AUXW), np.float32),
        })

    placement = []  # (core, k, b, t)
    for k, (w, chunk) in enumerate(slots):
        for core, (wt, b, t) in enumerate(chunk):
            placement.append((core, k, b, t))
            m = in_maps[core]
            na = int(num_atoms[b])
            m["cf"][cf_off[k]: cf_off[k] + 3 * w] = coords[b, :3 * w]
            m["cm"][cm_off[k]: cm_off[k] + w] = -(
                np.arange(w) < na).astype(np.float32)
            m["aux"][:, 3 * k: 3 * k + 3] = c3[b, t * P:(t + 1) * P]
            m["aux"][:, 3 * K + k] = (t * P + pidx < na)
    return in_maps, placement


_NC_CACHE = {}


def _get_program(widths):
    key = tuple(widths)
    if key not in _NC_CACHE:
        _NC_CACHE[key] = _build(list(widths))
    return _NC_CACHE[key]


def _reassemble(results, coords_shape, slots, placement):
    B, threeN = coords_shape[0], coords_shape[1]
    widths = [s[0] for s in slots]
    _, _, out_off, _, _, _ = _offsets(widths)
    out = np.zeros((B, threeN, threeN), np.float32)
    pidx = np.arange(P)
    a3 = np.arange(3)
    for (core, k, b, t) in placement:
        w = widths[k]
        blk = results[core]["out"][out_off[k]: out_off[k] + 384 * 3 * w]
        blk = blk.reshape(384, 3 * w)
        r = 384 * t
        out[b, r:r + 384, :3 * w] = blk
        # diagonal 3x3 blocks = -(row sums), exported via "dg"
        dg = results[core]["dg"][k].reshape(P, 3, 3)
        i3 = 3 * (t * P + pidx)
        rows = i3[:, None, None] + a3[None, :, None]
        cols = i3[:, None, None] + a3[None, None, :]
        out[b, rows, cols] = -dg
    return out


LAST_RUN = None  # BassKernelResults of the most recent kernel() call


def kernel(coords, num_atoms, _trace=False):
    global LAST_RUN
    coords = np.ascontiguousarray(np.asarray(coords, dtype=np.float32))
    na = np.asarray(num_atoms).astype(np.int64)
    slots = _plan(na)
    widths = [s[0] for s in slots]
    nc = _get_program(widths)
    in_maps, placement = _pack(coords, na, slots)
    LAST_RUN = run_bass_kernel_spmd(
        nc, in_maps, list(range(N_CORES)), trace=_trace,
        tmpdir=os.environ.get("TRACE_DIR") if _trace else None)
    return _reassemble(LAST_RUN.results, coords.shape, slots, placement)
